# revision 4
# baseline (speedup 1.0000x reference)
"""NeuroODE kernel for 8 Trainium2 NeuronCores.

Math: each Euler sub-step is y <- (a*I + b*P) y + g*ones, with P the
cyclic shift (roll by 1). Composing the 8 sub-steps of big step n gives a
9-tap circulant operator W_n; composing across big steps keeps the state
circulant in y0:

    y_n = C_n (*) y0 + s_n * ones

where C_n (tap vector, circular convolution) obeys C_{n+1} = W_n (*) C_n
and the forcing collapses to the scalar recurrence s_{n+1} = lam_n^8 s_n
+ g_n because P*ones = ones. The taps are a binomial bump centered at
~8*n*beta/(alpha+beta) <= ~170 with sigma <= ~13, so C_n is supported on
the first J taps (J=128 or 256 chosen from the actual weights). The whole
(2048, 2048) output is then

    Y = C_band (2048 x J) @ G_band (J x 2048) + s * ones^T,
    G_band[j, i] = y0[(i - j) mod 2048]

which is embarrassingly parallel over output rows: each of the 8 cores
computes 256 rows with J-deep matmuls. C_band/s are computed on host in
f64 (O(SAMPLE_NUM * J) work on 16 KB of data); all heavy lifting (the
16 MB output) happens on-device.
"""

import math

import numpy as np

SAMPLE_NUM = 2048
Y_NUM = 2048
STEP_N = 8
N_CORES = 8
ROWS_PER_CORE = SAMPLE_NUM // N_CORES  # 256

_COMPILED = {}  # J -> (nc, core_ids)


def _build_bass(J):
    import concourse.tile as tile
    from concourse import bacc, mybir

    f32 = mybir.dt.float32
    KC = J // 128  # contraction chunks of 128 (SBUF partition limit)
    nc = bacc.Bacc("TRN2", target_bir_lowering=False, debug=False,
                   num_devices=N_CORES)

    # ct[kc, k, m]: tap kc*128+k (contraction) for this core's output row m
    ct = nc.declare_dram_parameter("ct", [KC, 128, ROWS_PER_CORE], f32,
                                   isOutput=False)
    # g[kc, k, i] = y0[(i - (kc*128+k)) mod Y_NUM]
    g = nc.declare_dram_parameter("g", [KC, 128, Y_NUM], f32, isOutput=False)
    # s[p, mc]: forcing scalar for output row mc*128 + p
    s = nc.declare_dram_parameter("s", [128, ROWS_PER_CORE // 128], f32,
                                  isOutput=False)
    out = nc.declare_dram_parameter("out", [ROWS_PER_CORE, Y_NUM], f32,
                                    isOutput=True)

    NF = Y_NUM // 512  # moving-dim chunks
    NM = ROWS_PER_CORE // 128  # output row chunks

    with tile.TileContext(nc) as tc:
        with (
            tc.tile_pool(name="w", bufs=1) as wpool,
            tc.tile_pool(name="io", bufs=4) as iopool,
            tc.tile_pool(name="ps", bufs=4, space="PSUM") as pspool,
        ):
            s_sb = wpool.tile([128, NM], f32, tag="s")
            nc.sync.dma_start(s_sb[:], s[:])
            ct_sb = []
            for kc in range(KC):
                ctt = wpool.tile([128, ROWS_PER_CORE], f32, tag=f"ct{kc}")
                nc.sync.dma_start(ctt[:], ct[kc])
                ct_sb.append(ctt)
            g_sb = {}
            for f in range(NF):
                for kc in range(KC):
                    gt = wpool.tile([128, 512], f32, tag=f"g{f}_{kc}")
                    nc.sync.dma_start(gt[:], g[kc, :, f * 512:(f + 1) * 512])
                    g_sb[(f, kc)] = gt

            for f in range(NF):
                for mc in range(NM):
                    ps = pspool.tile([128, 512], f32, tag="ps")
                    for kc in range(KC):
                        nc.tensor.matmul(
                            ps[:],
                            ct_sb[kc][:, mc * 128:(mc + 1) * 128],
                            g_sb[(f, kc)][:],
                            start=(kc == 0),
                            stop=(kc == KC - 1),
                        )
                    ot = iopool.tile([128, 512], f32, tag="ot")
                    nc.vector.tensor_scalar_add(ot[:], ps[:], s_sb[:, mc:mc + 1])
                    nc.sync.dma_start(
                        out[mc * 128:(mc + 1) * 128, f * 512:(f + 1) * 512],
                        ot[:],
                    )

    nc.compile()
    return nc


def _get_compiled(J):
    if J not in _COMPILED:
        _COMPILED[J] = _build_bass(J)
    return _COMPILED[J]


def _host_prep(t, y0, weights, ratios):
    """f64 host math: tap matrix C (SAMPLE_NUM x J), forcing s, G band."""
    a = float(weights[0]) * float(ratios[0])
    b = float(weights[1]) * float(ratios[1])
    c = float(weights[2]) * float(ratios[2])

    t = t.astype(np.float32)
    steps_f32 = np.diff(t)                       # f32, as the reference
    sub_f32 = steps_f32 / np.float32(STEP_N)     # f32: big_step / step_n
    sub = sub_f32.astype(np.float64)
    alpha = 1.0 - sub * b
    beta = sub * a
    lam = alpha + beta

    # forcing: g_n accumulated over the 8 sub-steps with f32 time accrual
    # (tc advances in f32 exactly like the reference's carry)
    n = SAMPLE_NUM - 1
    gacc = np.zeros(n, dtype=np.float64)
    tc = t[:-1].copy()
    for _ in range(STEP_N):
        gacc = gacc * lam + sub * c * np.sin(tc.astype(np.float64))
        tc = tc + sub_f32
    s = np.zeros(SAMPLE_NUM, dtype=np.float64)
    lam8 = lam ** STEP_N
    for i in range(n):
        s[i + 1] = lam8[i] * s[i] + gacc[i]

    # taps: per big step the operator is sum_j C(8,j) alpha^(8-j) beta^j P^j
    binw = np.array([math.comb(STEP_N, j) for j in range(STEP_N + 1)])
    JMAX = 512
    C = np.zeros((SAMPLE_NUM, JMAX), dtype=np.float64)
    cur = np.zeros(JMAX, dtype=np.float64)
    cur[0] = 1.0
    C[0] = cur
    apow = alpha[:, None] ** np.arange(STEP_N, -1, -1.0)[None, :]
    bpow = beta[:, None] ** np.arange(0.0, STEP_N + 1.0)[None, :]
    wall = binw[None, :] * apow * bpow  # (n, 9)
    new = np.empty(JMAX, dtype=np.float64)
    for i in range(n):
        w = wall[i]
        new[:] = w[0] * cur
        for j in range(1, STEP_N + 1):
            new[j:] += w[j] * cur[:JMAX - j]
        cur, new = new, cur
        C[i + 1] = cur

    # choose band width J: smallest of {128, 256, 512} holding all the mass
    mass = np.abs(C).sum(axis=1)
    scale = np.maximum(mass, 1e-300)
    for J in (128, 256, 512):
        tail = np.abs(C[:, J - 8:J]).sum(axis=1) / scale
        if J == JMAX or tail.max() < 1e-12:
            break

    return C[:, :J].copy(), s, J


def kernel(t, y0, weights, ratios):
    t = np.asarray(t, dtype=np.float32)
    y0 = np.asarray(y0, dtype=np.float32)
    weights = np.asarray(weights, dtype=np.float32)
    ratios = np.asarray(ratios, dtype=np.float32)
    assert t.shape == (SAMPLE_NUM,) and y0.shape == (Y_NUM,)

    C, s, J = _host_prep(t, y0, weights, ratios)

    # G_band[j, i] = y0[(i - j) mod Y_NUM]
    idx = (np.arange(Y_NUM)[None, :] - np.arange(J)[:, None]) % Y_NUM
    G = np.ascontiguousarray(
        y0[idx].astype(np.float32).reshape(J // 128, 128, Y_NUM))

    Cf = C.astype(np.float32)
    sf = s.astype(np.float32)

    nc = _get_compiled(J)
    core_ids = list(range(N_CORES))
    in_maps = []
    for q in core_ids:
        rows = slice(q * ROWS_PER_CORE, (q + 1) * ROWS_PER_CORE)
        ctq = np.ascontiguousarray(
            Cf[rows].T.reshape(J // 128, 128, ROWS_PER_CORE))  # (KC, 128, 256)
        sq = np.ascontiguousarray(
            sf[rows].reshape(ROWS_PER_CORE // 128, 128).T)  # (128, NM)
        in_maps.append({"ct": ctq, "g": G, "s": sq})

    from concourse.bass_utils import run_bass_kernel_spmd
    res = run_bass_kernel_spmd(nc, in_maps, core_ids)
    return np.concatenate([res.results[q]["out"] for q in core_ids], axis=0)


# revision 6
# speedup vs baseline: 1.5806x; 1.5806x over previous
"""NeuroODE kernel for 8 Trainium2 NeuronCores.

Math: each Euler sub-step is y <- (alpha*I + beta*P) y + gamma*ones, with
P the cyclic shift (roll by 1). Composing the 8 sub-steps of big step n
gives a 9-tap circulant operator W_n; composing across big steps keeps the
state circulant in y0:

    y_n = C_n (*) y0 + s_n * ones

where C_n (tap vector, circular convolution) obeys C_{n+1} = W_n (*) C_n
and the forcing collapses to the scalar recurrence s_{n+1} = lam_n^8 s_n
+ g_n because P*ones = ones. The taps are a binomial bump centered at
~8*n*beta/(alpha+beta) with small sigma, so C_n is supported on the first
TAPS taps (TAPS chosen from the actual weights at runtime). The whole
(2048, 2048) output is then one banded matmul

    Y[n, i] = sum_k C[n, k] * y0[(i - k) mod 2048] + s_n

which is embarrassingly parallel over output rows: each of the 8 cores
computes 256 rows. The s_n bias is folded into the matmul as an extra
contraction row (ct row J-1 = s, g row J-1 = ones), so the device kernel
is pure matmul (float32r, full f32 precision) + DMA. C/s are computed on
host in f64 (O(SAMPLE_NUM * TAPS) work on 16 KB of data); all heavy
lifting (the 16 MB output) happens on-device.
"""

import math

import numpy as np

SAMPLE_NUM = 2048
Y_NUM = 2048
STEP_N = 8
N_CORES = 8
ROWS_PER_CORE = SAMPLE_NUM // N_CORES  # 256

_COMPILED = {}  # J -> nc


def _build_bass(J):
    import concourse.tile as tile
    from concourse import bacc, mybir

    f32 = mybir.dt.float32
    f32r = mybir.dt.float32r
    KC = J // 128  # contraction chunks of 128 (SBUF partition limit)
    NF = Y_NUM // 512  # moving-dim chunks of 512
    NM = ROWS_PER_CORE // 128  # output row chunks

    nc = bacc.Bacc("TRN2", target_bir_lowering=False, debug=False,
                   num_devices=N_CORES)

    # ct[kc, k, m]: coefficient for tap kc*128+k, output row m; the very
    # last (kc, k) row holds the forcing bias s_m instead of a tap.
    ct = nc.declare_dram_parameter("ct", [KC, 128, ROWS_PER_CORE], f32r,
                                   isOutput=False)
    # g[kc, k, i] = y0[(i - (kc*128+k)) mod Y_NUM]; last row all-ones.
    g = nc.declare_dram_parameter("g", [KC, 128, Y_NUM], f32r, isOutput=False)
    out = nc.declare_dram_parameter("out", [ROWS_PER_CORE, Y_NUM], f32,
                                    isOutput=True)

    with tile.TileContext(nc) as tc:
        with (
            tc.tile_pool(name="w", bufs=1) as wpool,
            tc.tile_pool(name="io", bufs=4) as iopool,
            tc.tile_pool(name="ps", bufs=8, space="PSUM") as pspool,
        ):
            ct_sb = []
            for kc in range(KC):
                ctt = wpool.tile([128, ROWS_PER_CORE], f32r, tag=f"ct{kc}",
                                 name=f"ct{kc}")
                nc.sync.dma_start(ctt[:], ct[kc])
                ct_sb.append(ctt)
            g_sb = {}
            for f in range(NF):
                for kc in range(KC):
                    gt = wpool.tile([128, 512], f32r, tag=f"g{f}_{kc}",
                                    name=f"g{f}_{kc}")
                    nc.sync.dma_start(gt[:], g[kc, :, f * 512:(f + 1) * 512])
                    g_sb[(f, kc)] = gt

            for mc in range(NM):
                for f in range(NF):
                    ps = pspool.tile([128, 512], f32, tag="ps", name="ps")
                    for kc in range(KC):
                        nc.tensor.matmul(
                            ps[:],
                            ct_sb[kc][:, mc * 128:(mc + 1) * 128],
                            g_sb[(f, kc)][:],
                            start=(kc == 0),
                            stop=(kc == KC - 1),
                        )
                    ot = iopool.tile([128, 512], f32, tag="ot", name="ot")
                    if (mc * NF + f) % 2 == 0:
                        nc.vector.tensor_copy(ot[:], ps[:])
                    else:
                        nc.scalar.copy(ot[:], ps[:])
                    nc.sync.dma_start(
                        out[mc * 128:(mc + 1) * 128, f * 512:(f + 1) * 512],
                        ot[:],
                    )

    nc.compile()
    return nc


def _get_compiled(J):
    if J not in _COMPILED:
        _COMPILED[J] = _build_bass(J)
    return _COMPILED[J]


def _host_prep(t, y0, weights, ratios):
    """f64 host math: tap matrix C (SAMPLE_NUM x TAPS) and forcing s."""
    a = float(weights[0]) * float(ratios[0])
    b = float(weights[1]) * float(ratios[1])
    c = float(weights[2]) * float(ratios[2])

    t = t.astype(np.float32)
    steps_f32 = np.diff(t)                       # f32, as the reference
    sub_f32 = steps_f32 / np.float32(STEP_N)     # f32: big_step / step_n
    sub = sub_f32.astype(np.float64)
    alpha = 1.0 - sub * b
    beta = sub * a
    lam = alpha + beta

    # forcing: g_n accumulated over the 8 sub-steps with f32 time accrual
    # (tc advances in f32 exactly like the reference's scan carry)
    n = SAMPLE_NUM - 1
    gacc = np.zeros(n, dtype=np.float64)
    tc = t[:-1].copy()
    for _ in range(STEP_N):
        gacc = gacc * lam + sub * c * np.sin(tc.astype(np.float64))
        tc = tc + sub_f32
    s = np.zeros(SAMPLE_NUM, dtype=np.float64)
    lam8 = lam ** STEP_N
    for i in range(n):
        s[i + 1] = lam8[i] * s[i] + gacc[i]

    # taps: per big step the operator is sum_j C(8,j) alpha^(8-j) beta^j P^j
    binw = np.array([math.comb(STEP_N, j) for j in range(STEP_N + 1)])
    JMAX = 512
    C = np.zeros((SAMPLE_NUM, JMAX), dtype=np.float64)
    cur = np.zeros(JMAX, dtype=np.float64)
    cur[0] = 1.0
    C[0] = cur
    apow = alpha[:, None] ** np.arange(STEP_N, -1, -1.0)[None, :]
    bpow = beta[:, None] ** np.arange(0.0, STEP_N + 1.0)[None, :]
    wall = binw[None, :] * apow * bpow  # (n, 9)
    new = np.empty(JMAX, dtype=np.float64)
    for i in range(n):
        w = wall[i]
        new[:] = w[0] * cur
        for j in range(1, STEP_N + 1):
            new[j:] += w[j] * cur[:JMAX - j]
        cur, new = new, cur
        C[i + 1] = cur

    # band width: smallest J in {128, 256, 512} such that dropping taps
    # >= J-1 (the last row is repurposed for the bias) is negligible
    mass = np.maximum(np.abs(C).sum(axis=1), 1e-300)
    for J in (128, 256, 512):
        tail = np.abs(C[:, J - 9:J]).sum(axis=1) / mass
        if J == JMAX or tail.max() < 1e-12:
            break

    return C[:, :J - 1].copy(), s, J


def kernel(t, y0, weights, ratios):
    t = np.asarray(t, dtype=np.float32)
    y0 = np.asarray(y0, dtype=np.float32)
    weights = np.asarray(weights, dtype=np.float32)
    ratios = np.asarray(ratios, dtype=np.float32)
    assert t.shape == (SAMPLE_NUM,) and y0.shape == (Y_NUM,)

    C, s, J = _host_prep(t, y0, weights, ratios)
    TAPS = J - 1

    # g[k, i] = y0[(i - k) mod Y_NUM] for k < TAPS; row TAPS = ones (bias)
    idx = (np.arange(Y_NUM)[None, :] - np.arange(TAPS)[:, None]) % Y_NUM
    G = np.empty((J, Y_NUM), dtype=np.float32)
    G[:TAPS] = y0[idx]
    G[TAPS] = 1.0
    G = np.ascontiguousarray(G.reshape(J // 128, 128, Y_NUM))

    Cf = C.astype(np.float32)    # (SAMPLE_NUM, TAPS)
    sf = s.astype(np.float32)

    nc = _get_compiled(J)
    core_ids = list(range(N_CORES))
    in_maps = []
    for q in core_ids:
        rows = slice(q * ROWS_PER_CORE, (q + 1) * ROWS_PER_CORE)
        ctq = np.empty((J, ROWS_PER_CORE), dtype=np.float32)
        ctq[:TAPS] = Cf[rows].T
        ctq[TAPS] = sf[rows]
        ctq = np.ascontiguousarray(ctq.reshape(J // 128, 128, ROWS_PER_CORE))
        in_maps.append({"ct": ctq, "g": G})

    from concourse.bass_utils import run_bass_kernel_spmd
    res = run_bass_kernel_spmd(nc, in_maps, core_ids)
    return np.concatenate([res.results[q]["out"] for q in core_ids], axis=0)
